# revision 23
# baseline (speedup 1.0000x reference)
"""HSTU block kernel for 8 TRN2 NeuronCores (nn_HSTU_66279935312625).

Sharding: 2 cores per batch (B=4). Core pair splits the 16 q-tiles
(128 rows each) causally balanced: g=0 owns tiles {15,13,11,9,6,4,2,0},
g=1 owns {14,12,10,8,7,5,3,1}, ordered descending-extent within two
512-row q-units so the moving q-stream width shrinks with k-block
index. Program k-extents per unit position: unit0 (16,14,12,10), unit1
(8,6,4,2); k-blocks iterate descending so PSUM AV accumulation uses one
bank-wide start (per-element has_written handles growing widths).
72 q128xk128 blocks per head vs 136 full-causal (optimum 68).

K/V are projected for a contiguous half of rows per core and exchanged
with the pair core via two DRAM AllGathers (K, then V), overlapped with
the Q/U projections; assembly DMAs ride the otherwise-idle gpsimd queue.

All matmuls bf16 (fp32 PSUM). Attention S and AV are emitted as
adjacent pairs on disjoint PE row/col groups (2-head packing) so the
16 32x32 sub-arrays run both heads concurrently; the emission is
software-pipelined (S-pair(kb) -> AV-pair(kb+1's prev) -> sigmoids) to
keep pairs adjacent in the PE queue. Biases fold into activation bias
APs / vector adds. LayerNorm stats are deferred after each unit's
attention (frees PSUM banks), gamma/beta apply via scalar activation
scale/bias APs, and the u0 tail (LN + out-proj, 2 PSUM banks) overlaps
u1 attention (6 banks). Final out = gated @ W_out + b_out + x.
"""
import numpy as np

import concourse.bacc as bacc
import concourse.tile as tile
from concourse import mybir
from concourse.bass_utils import run_bass_kernel_spmd
from concourse.tile_rust import add_dep_helper

F32 = mybir.dt.float32
F32R = mybir.dt.float32r
BF16 = mybir.dt.bfloat16
AF = mybir.ActivationFunctionType

B, L, D, H, HD = 4, 2048, 1024, 16, 64
OWN = 1024          # rows owned per core
UQ = 512            # rows per q-unit
EXT0 = (16, 14, 12, 10)   # program k-extent per q-tile position, unit 0
EXT1 = (8, 6, 4, 2)       # unit 1
SCALE = HD ** -0.5
LN_EPS = 1e-8
NCORES = 8

TILES0 = (15, 13, 11, 9, 6, 4, 2, 0)
TILES1 = (14, 12, 10, 8, 7, 5, 3, 1)

_CACHED = {}


def _build():
    nc = bacc.Bacc("TRN2", target_bir_lowering=False, debug=False)

    xkvT = nc.dram_tensor("xkvT", [D, OWN], BF16, kind="ExternalInput").ap()
    xqT = nc.dram_tensor("xqT", [D, OWN], BF16, kind="ExternalInput").ap()
    xq = nc.dram_tensor("xq", [OWN, D], F32, kind="ExternalInput").ap()
    wproj = nc.dram_tensor("wproj", [D, 4 * D], BF16, kind="ExternalInput").ap()
    wout = nc.dram_tensor("wout", [D, D], BF16, kind="ExternalInput").ap()
    cosk = nc.dram_tensor("cosk", [128, OWN], BF16, kind="ExternalInput").ap()
    sink = nc.dram_tensor("sink", [128, OWN], BF16, kind="ExternalInput").ap()
    cosq = nc.dram_tensor("cosq", [128, OWN], BF16, kind="ExternalInput").ap()
    sinq = nc.dram_tensor("sinq", [128, OWN], BF16, kind="ExternalInput").ap()
    p2 = nc.dram_tensor("p2", [128, 128], BF16, kind="ExternalInput").ap()
    maskT = nc.dram_tensor("maskT", [128, 2, 8, 128], BF16,
                           kind="ExternalInput").ap()
    bcolT = nc.dram_tensor("bcolT", [128, 32], F32, kind="ExternalInput").ap()
    gbT = nc.dram_tensor("gbT", [128, 16], F32, kind="ExternalInput").ap()
    vbias = nc.dram_tensor("vbias", [128, D], F32, kind="ExternalInput").ap()
    boutb = nc.dram_tensor("boutb", [128, D], F32, kind="ExternalInput").ap()
    ones128 = nc.dram_tensor("ones128", [128, 1], F32R, kind="ExternalInput").ap()
    onesrow = nc.dram_tensor("onesrow", [1, 128], F32R, kind="ExternalInput").ap()
    out = nc.dram_tensor("out", [OWN, D], F32, kind="ExternalOutput").ap()
    # pair-allgather bounce buffers
    kout = nc.dram_tensor("kout", [128, 8192], BF16, kind="Internal").ap()
    vout = nc.dram_tensor("vout", [128, 8192], BF16, kind="Internal").ap()
    kg = nc.dram_tensor("kg", [2, 128, 8192], BF16, kind="Internal").ap()
    vg = nc.dram_tensor("vg", [2, 128, 8192], BF16, kind="Internal").ap()

    wp3 = wproj.rearrange("(t ki) n -> ki t n", ki=128)   # [128, 8, 4096]
    wo3 = wout.rearrange("(t ki) n -> ki t n", ki=128)    # [128, 8, 1024]
    xkv3 = xkvT.rearrange("(t ki) n -> ki t n", ki=128)   # [128, 8, 1024]
    xq3 = xqT.rearrange("(t ki) n -> ki t n", ki=128)     # [128, 8, 1024]
    RG = [[0, 1], [2, 3], [4, 5], [6, 7]]

    with tile.TileContext(nc) as tc:
        with (
            tc.tile_pool(name="const", bufs=1) as cpool,
            tc.tile_pool(name="big", bufs=1) as big,
        ):
            p2sb = cpool.tile([128, 128], BF16)
            nc.sync.dma_start(p2sb[:], p2)
            bcsb = cpool.tile([128, 32], F32)
            nc.sync.dma_start(bcsb[:], bcolT)
            gbsb = cpool.tile([128, 16], F32)
            nc.sync.dma_start(gbsb[:], gbT)
            vbsb = cpool.tile([128, D], F32)
            nc.sync.dma_start(vbsb[:], vbias)
            bosb = cpool.tile([128, D], F32)
            nc.sync.dma_start(bosb[:], boutb)
            o128 = cpool.tile([128, 1], F32R)
            nc.sync.dma_start(o128[:], ones128)
            orow = cpool.tile([1, 128], F32R)
            nc.sync.dma_start(orow[:], onesrow)
            epsb = cpool.tile([1, 1], F32)
            nc.vector.memset(epsb[:], LN_EPS)

            krot = big.tile([128, 8, L], BF16)      # K_rot^T  [2-head col tiles]
            v16 = big.tile([128, 16, D], BF16)      # V natural [row tiles]
            qrot = big.tile([128, 8, OWN], BF16)    # Q_rot^T
            silu16 = big.tile([128, 8, OWN], BF16)  # silu(U)^T

            # ================= phase 1: K and V (own half) =================
            with (
                tc.tile_pool(name="rope", bufs=3) as rope,
                tc.tile_pool(name="ppj", bufs=4, space="PSUM") as ppj,
                tc.tile_pool(name="prt", bufs=2, space="PSUM") as prt,
            ):
                def rope_chain(psP, bias_sl, cos_sl, sin_sl, dst, rp, pp):
                    # dst(bf16) = (psP+b)*cos + rotate_half(psP+b)*sin
                    t16 = rp.tile([128, UQ], BF16, tag="t16")
                    nc.scalar.activation(t16[:], psP[:], AF.Identity,
                                         bias=bias_sl)
                    psR = pp.tile([128, UQ], F32, tag="psR")
                    nc.tensor.matmul(psR[:], p2sb[:], t16[:], start=True,
                                     stop=True)
                    tcos = rp.tile([128, UQ], BF16, tag="tcos")
                    nc.vector.tensor_mul(tcos[:], t16[:], cos_sl)
                    tsin = rp.tile([128, UQ], BF16, tag="tsin")
                    nc.vector.tensor_mul(tsin[:], psR[:], sin_sl)
                    nc.vector.tensor_add(dst, tcos[:], tsin[:])

                kv_scope = tc.tile_pool(name="ph1", bufs=1)
                ph1 = kv_scope.__enter__()
                wr_scope = tc.tile_pool(name="wring", bufs=3)
                wring = wr_scope.__enter__()
                st_scope = tc.tile_pool(name="stage", bufs=4)
                stage = st_scope.__enter__()

                xh = ph1.tile([128, 8, 1024], BF16, tag="xh")
                nc.sync.dma_start(xh[:], xkv3)
                cksb = ph1.tile([128, 1024], BF16, tag="cksb")
                nc.sync.dma_start(cksb[:], cosk)
                sksb = ph1.tile([128, 1024], BF16, tag="sksb")
                nc.sync.dma_start(sksb[:], sink)

                for ct in range(8):
                    c0 = 3 * D + 128 * ct
                    wk = wring.tile([128, 8, 128], BF16, tag="wk")
                    nc.sync.dma_start(wk[:], wp3[:, :, c0:c0 + 128])
                    for r in range(2):
                        ps = ppj.tile([128, UQ], F32, tag="ps")
                        for t in range(8):
                            nc.tensor.matmul(
                                ps[:], wk[:, t, :],
                                xh[:, t, r * UQ:(r + 1) * UQ],
                                start=(t == 0), stop=(t == 7))
                        off = r * UQ
                        kst = stage.tile([128, UQ], BF16, tag="kst")
                        rope_chain(ps, bcsb[:, 24 + ct:25 + ct],
                                   cksb[:, off:off + UQ],
                                   sksb[:, off:off + UQ],
                                   kst[:], rope, prt)
                        nc.sync.dma_start(
                            kout[:, 1024 * ct + off:1024 * ct + off + UQ],
                            kst[:])

                # K halves exchange; assembly DMAs on the idle gpsimd queue
                nc.gpsimd.collective_compute(
                    "AllGather", mybir.AluOpType.bypass, replica_groups=RG,
                    ins=[kout.opt()], outs=[kg.opt()])
                for gi in range(2):
                    kpart = kg[gi, :, :].rearrange("p (c n) -> p c n", c=8)
                    nc.gpsimd.dma_start(
                        krot[:, :, 1024 * gi:1024 * (gi + 1)], kpart)

                for vh in range(2):
                    v0 = D + UQ * vh
                    wvh = wring.tile([128, 8, UQ], BF16, tag="wv", bufs=1)
                    nc.sync.dma_start(wvh[:], wp3[:, :, v0:v0 + UQ])
                    for rv in range(8):
                        pv = ppj.tile([128, UQ], F32, tag="ps")
                        for t in range(8):
                            nc.tensor.matmul(
                                pv[:], xh[:, t, 128 * rv:128 * (rv + 1)],
                                wvh[:, t, :], start=(t == 0), stop=(t == 7))
                        vst = stage.tile([128, UQ], BF16, tag="vst")
                        nc.vector.tensor_add(
                            vst[:], pv[:], vbsb[:, UQ * vh:UQ * (vh + 1)])
                        nc.sync.dma_start(
                            vout[:, 1024 * rv + UQ * vh:
                                 1024 * rv + UQ * (vh + 1)],
                            vst[:])

                nc.gpsimd.collective_compute(
                    "AllGather", mybir.AluOpType.bypass, replica_groups=RG,
                    ins=[vout.opt()], outs=[vg.opt()])
                for gi in range(2):
                    vpart = vg[gi, :, :].rearrange("p (c n) -> p c n", c=8)
                    nc.gpsimd.dma_start(v16[:, 8 * gi:8 * (gi + 1), :], vpart)

                st_scope.__exit__(None, None, None)
                wr_scope.__exit__(None, None, None)
                kv_scope.__exit__(None, None, None)

                # ============= phase 1a: Q and U =============
                with (
                    tc.tile_pool(name="ph1a", bufs=1) as ph1a,
                    tc.tile_pool(name="wring2", bufs=2) as wring2,
                ):
                    xqsb = ph1a.tile([128, 8, OWN], BF16)
                    nc.sync.dma_start(xqsb[:], xq3)
                    cqsb = ph1a.tile([128, OWN], BF16)
                    nc.sync.dma_start(cqsb[:], cosq)
                    sqsb = ph1a.tile([128, OWN], BF16)
                    nc.sync.dma_start(sqsb[:], sinq)

                    for ct in range(8):
                        wu = wring2.tile([128, 8, 128], BF16, tag="wu")
                        nc.sync.dma_start(wu[:], wp3[:, :, 128 * ct:128 * (ct + 1)])
                        q0 = 2 * D + 128 * ct
                        wq = wring2.tile([128, 8, 128], BF16, tag="wq")
                        nc.sync.dma_start(wq[:], wp3[:, :, q0:q0 + 128])
                        for r in range(2):
                            sl = slice(r * UQ, (r + 1) * UQ)
                            psu = ppj.tile([128, UQ], F32, tag="ps")
                            for t in range(8):
                                nc.tensor.matmul(psu[:], wu[:, t, :],
                                                 xqsb[:, t, sl],
                                                 start=(t == 0), stop=(t == 7))
                            nc.scalar.activation(silu16[:, ct, sl], psu[:],
                                                 AF.Silu,
                                                 bias=bcsb[:, ct:ct + 1])
                            psq = ppj.tile([128, UQ], F32, tag="ps")
                            for t in range(8):
                                nc.tensor.matmul(psq[:], wq[:, t, :],
                                                 xqsb[:, t, sl],
                                                 start=(t == 0), stop=(t == 7))
                            rope_chain(psq, bcsb[:, 16 + ct:17 + ct],
                                       cqsb[:, sl], sqsb[:, sl],
                                       qrot[:, ct, sl], rope, prt)

            # ================= phase 2: attention + tail =================
            with (
                tc.tile_pool(name="ph2", bufs=1) as ph2,
                tc.tile_pool(name="mring", bufs=2) as mring,
                tc.tile_pool(name="aring", bufs=4) as aring,
                tc.tile_pool(name="sqring", bufs=2) as sqring,
                tc.tile_pool(name="gring", bufs=2) as gring,
                tc.tile_pool(name="oring", bufs=2) as oring,
                tc.tile_pool(name="psS", bufs=2, space="PSUM") as psSp,
                tc.tile_pool(name="psO", bufs=2, space="PSUM") as psOp,
            ):
                wosb = ph2.tile([128, 8, D], BF16)
                nc.sync.dma_start(wosb[:], wo3)
                attnT = ph2.tile([128, 8, UQ], F32R)
                gated = ph2.tile([128, 8, UQ], BF16)
                statr = ph2.tile([1, 4, UQ], F32R)

                for u in range(2):
                    EXT = EXT0 if u == 0 else EXT1
                    msb = mring.tile([128, 8, 128], BF16, tag="msb")
                    nc.sync.dma_start(msb[:], maskT[:, u, :, :])
                    for hp in range(8):
                        psO = psOp.tile([128, UQ], F32, tag="psO")
                        q0c = u * UQ
                        prev = None
                        # chain attention matmuls in emission order so the
                        # scheduler keeps row/col-group pairs adjacent in the
                        # PE queue (sub-array concurrency).
                        last_mm = [None]

                        def chain(mm):
                            if last_mm[0] is not None:
                                add_dep_helper(mm.ins, last_mm[0].ins,
                                               reason="pe-pair-order")
                            last_mm[0] = mm

                        def av_pair(pr):
                            kbp, wp, aABp = pr
                            st = kbp == EXT[0] - 1
                            sp = kbp == 0
                            chain(nc.tensor.matmul(
                                psO[0:64, 0:wp],
                                v16[:, kbp, 128 * hp:128 * hp + 64],
                                aABp[:, 0, 0:wp], start=st, stop=sp,
                                tile_position=(0, 0)))
                            chain(nc.tensor.matmul(
                                psO[64:128, 0:wp],
                                v16[:, kbp, 128 * hp + 64:128 * (hp + 1)],
                                aABp[:, 1, 0:wp], start=st, stop=sp,
                                tile_position=(0, 64)))

                        for kb in range(EXT[0] - 1, -1, -1):
                            nP = sum(1 for e in EXT if e > kb)
                            w = 128 * nP
                            psAB = psSp.tile([128, 2, UQ], F32, tag="psS")
                            kbs = slice(128 * kb, 128 * (kb + 1))
                            chain(nc.tensor.matmul(
                                psAB[:, 0, 0:w], krot[0:64, hp, kbs],
                                qrot[0:64, hp, q0c:q0c + w],
                                start=True, stop=True))
                            chain(nc.tensor.matmul(
                                psAB[:, 1, 0:w], krot[64:128, hp, kbs],
                                qrot[64:128, hp, q0c:q0c + w],
                                start=True, stop=True))
                            if prev is not None:
                                av_pair(prev)
                            aAB = aring.tile([128, 2, UQ], BF16, tag="aAB")
                            nc.scalar.activation(aAB[:, :, 0:w],
                                                 psAB[:, :, 0:w],
                                                 AF.Sigmoid, scale=SCALE)
                            pl = nP - 1
                            if kb >= EXT[pl] - 2:
                                m = 2 * pl + (1 if kb == EXT[pl] - 2 else 0)
                                nc.vector.tensor_mul(
                                    aAB[:, 0, w - 128:w],
                                    aAB[:, 0, w - 128:w], msb[:, m, :])
                                nc.vector.tensor_mul(
                                    aAB[:, 1, w - 128:w],
                                    aAB[:, 1, w - 128:w], msb[:, m, :])
                            prev = (kb, w, aAB)
                        av_pair(prev)
                        nc.vector.tensor_copy(attnT[:, hp, :], psO[:])

                    # ---- deferred LN stats ----
                    with tc.tile_pool(name=f"psT{u}", bufs=2,
                                      space="PSUM") as psTp:
                        psSum = psTp.tile([1, UQ], F32, tag="st")
                        psSq = psTp.tile([1, UQ], F32, tag="st")
                        for c in range(8):
                            sq = sqring.tile([128, UQ], F32R, tag="sq")
                            nc.vector.tensor_mul(sq[:], attnT[:, c, :],
                                                 attnT[:, c, :])
                            nc.tensor.matmul(psSum[:], o128[:],
                                             attnT[:, c, :],
                                             start=(c == 0), stop=(c == 7))
                            nc.tensor.matmul(psSq[:], o128[:], sq[:],
                                             start=(c == 0), stop=(c == 7))
                        mu = statr[0:1, 0, :]
                        nc.vector.tensor_scalar_mul(mu, psSum[:], 1.0 / D)
                        m2 = statr[0:1, 1, :]
                        nc.vector.tensor_scalar_mul(m2, psSq[:], 1.0 / D)
                        musq = statr[0:1, 2, :]
                        nc.vector.tensor_mul(musq, mu, mu)
                        varr = statr[0:1, 1, :]
                        nc.vector.tensor_sub(varr, m2, musq)
                        rstd = statr[0:1, 3, :]
                        nc.scalar.activation(rstd, varr, AF.Sqrt, bias=epsb[:])
                        with nc.allow_low_precision("f32r rstd for matmul"):
                            nc.vector.reciprocal(rstd, rstd)
                        nmr = statr[0:1, 2, :]
                        nc.vector.tensor_mul(nmr, mu, rstd)
                        nc.vector.tensor_scalar_mul(nmr, nmr, -1.0)

                    # ---- LN apply + gating ----
                    with tc.tile_pool(name=f"psG{u}", bufs=2,
                                      space="PSUM") as psGp:
                        psRB = psGp.tile([128, UQ], F32, tag="bc")
                        nc.tensor.matmul(psRB[:], orow[:], statr[0:1, 3, :],
                                         start=True, stop=True)
                        psNB = psGp.tile([128, UQ], F32, tag="bc")
                        nc.tensor.matmul(psNB[:], orow[:], statr[0:1, 2, :],
                                         start=True, stop=True)
                        for c in range(8):
                            g1 = gring.tile([128, UQ], F32, tag="g1")
                            nc.vector.tensor_mul(g1[:], attnT[:, c, :],
                                                 psRB[:])
                            g2 = gring.tile([128, UQ], F32, tag="g2")
                            nc.vector.tensor_add(g2[:], g1[:], psNB[:])
                            g3 = gring.tile([128, UQ], F32, tag="g3")
                            nc.scalar.activation(g3[:], g2[:], AF.Identity,
                                                 scale=gbsb[:, c:c + 1],
                                                 bias=gbsb[:, 8 + c:9 + c])
                            nc.vector.tensor_mul(
                                gated[:, c, :], g3[:],
                                silu16[:, c, u * UQ:(u + 1) * UQ])

                    # ---- out projection (2 PSUM banks) ----
                    with tc.tile_pool(name=f"psP{u}", bufs=2,
                                      space="PSUM") as psPp:
                        for rw in range(4):
                            r0 = u * UQ + 128 * rw
                            xqn = oring.tile([128, D], F32, tag="xqn")
                            nc.sync.dma_start(xqn[:], xq[r0:r0 + 128, :])
                            pbs = [psPp.tile([128, UQ], F32, tag="po",
                                             name=f"po{u}_{rw}_{i}")
                                   for i in range(2)]
                            for c in range(8):
                                st = gated[:, c, 128 * rw:128 * (rw + 1)]
                                for oh in range(2):
                                    nc.tensor.matmul(
                                        pbs[oh][:], st,
                                        wosb[:, c, UQ * oh:UQ * (oh + 1)],
                                        start=(c == 0), stop=(c == 7))
                            for oh in range(2):
                                ohs = slice(UQ * oh, UQ * (oh + 1))
                                osb = oring.tile([128, UQ], F32, tag="osb")
                                nc.vector.tensor_add(osb[:], pbs[oh][:],
                                                     xqn[:, ohs])
                                osb2 = oring.tile([128, UQ], F32, tag="osb2")
                                nc.gpsimd.tensor_add(osb2[:], osb[:],
                                                     bosb[:, ohs])
                                nc.sync.dma_start(out[r0:r0 + 128, ohs],
                                                  osb2[:])
    nc.finalize()
    return nc


def _host_prep(x, attn_mask, W_proj, b_proj, ln_gamma, ln_beta, W_out, b_out):
    """Build the 8 per-core input maps."""
    import ml_dtypes
    bf16 = ml_dtypes.bfloat16

    x = np.asarray(x, dtype=np.float32)
    attn_mask = np.asarray(attn_mask)
    W_proj = np.ascontiguousarray(np.asarray(W_proj, dtype=np.float32))
    W_out = np.ascontiguousarray(np.asarray(W_out, dtype=np.float32))
    b_proj = np.asarray(b_proj, dtype=np.float32)
    b_out = np.asarray(b_out, dtype=np.float32)
    ln_gamma = np.asarray(ln_gamma, dtype=np.float32)
    ln_beta = np.asarray(ln_beta, dtype=np.float32)

    inv = 1.0 / (10000.0 ** (np.arange(0, HD, 2, dtype=np.float64) / HD))
    ang = np.outer(inv, np.arange(L, dtype=np.float64))       # [32, L]
    c64 = np.concatenate([np.cos(ang), np.cos(ang)], 0)
    s64 = np.concatenate([np.sin(ang), np.sin(ang)], 0)
    cosk = np.concatenate([c64, c64], 0).astype(np.float32)   # [128, L]
    sink = np.concatenate([s64, s64], 0).astype(np.float32)

    p2 = np.zeros((128, 128), dtype=np.float32)
    for base in (0, 64):
        for m in range(32):
            p2[base + m + 32, base + m] = -1.0
        for m in range(32, 64):
            p2[base + m - 32, base + m] = 1.0

    # per-partition column biases [128, 32]: sections U,V,Q,K x 8 chunks
    bcolT = np.empty((128, 32), dtype=np.float32)
    for s in range(4):
        for c in range(8):
            bcolT[:, 8 * s + c] = b_proj[s * D + 128 * c:s * D + 128 * (c + 1)]
    gbT = np.empty((128, 16), dtype=np.float32)
    for c in range(8):
        gbT[:, c] = ln_gamma[128 * c:128 * (c + 1)]
        gbT[:, 8 + c] = ln_beta[128 * c:128 * (c + 1)]
    vbias = np.broadcast_to(b_proj[D:2 * D], (128, D))
    boutb = np.broadcast_to(b_out, (128, D))

    shared = dict(
        wproj=W_proj.astype(bf16), wout=W_out.astype(bf16),
        p2=p2.astype(bf16),
        bcolT=bcolT, gbT=gbT,
        vbias=np.ascontiguousarray(vbias),
        boutb=np.ascontiguousarray(boutb),
        ones128=np.ones((128, 1), np.float32),
        onesrow=np.ones((1, 128), np.float32),
    )

    exts = (EXT0, EXT1)
    in_maps = []
    for cid in range(NCORES):
        b, g = divmod(cid, 2)
        tiles = TILES0 if g == 0 else TILES1
        own = np.concatenate([np.arange(128 * t, 128 * (t + 1))
                              for t in tiles])
        xb = x[b]
        xqc = np.ascontiguousarray(xb[own])
        half = slice(OWN * g, OWN * (g + 1))
        m = dict(shared)
        m["xkvT"] = np.ascontiguousarray(xb[half].T).astype(bf16)
        m["cosk"] = np.ascontiguousarray(cosk[:, half]).astype(bf16)
        m["sink"] = np.ascontiguousarray(sink[:, half]).astype(bf16)
        m["xqT"] = np.ascontiguousarray(xqc.T).astype(bf16)
        m["xq"] = xqc
        m["cosq"] = np.ascontiguousarray(cosk[:, own]).astype(bf16)
        m["sinq"] = np.ascontiguousarray(sink[:, own]).astype(bf16)
        am = attn_mask[b]
        mk = np.zeros((128, 2, 8, 128), dtype=np.float32)
        for uu in range(2):
            for p in range(4):
                t = tiles[4 * uu + p]
                qrows = slice(128 * t, 128 * (t + 1))
                for w01 in range(2):
                    kb = exts[uu][p] - 1 - w01
                    mk[:, uu, 2 * p + w01, :] = \
                        am[qrows, 128 * kb:128 * (kb + 1)].T
        m["maskT"] = mk.astype(bf16)
        in_maps.append(m)
    return in_maps


def kernel(**inputs):
    if "nc" not in _CACHED:
        _CACHED["nc"] = _build()
    nc = _CACHED["nc"]
    in_maps = _host_prep(**inputs)
    res = run_bass_kernel_spmd(nc, in_maps, list(range(NCORES)))
    globals()["_LAST_RESULTS"] = res
    full = np.empty((B, L, D), dtype=np.float32)
    for cid in range(NCORES):
        b, g = divmod(cid, 2)
        tiles = TILES0 if g == 0 else TILES1
        o = res.results[cid]["out"]
        for i, t in enumerate(tiles):
            full[b, 128 * t:128 * (t + 1)] = o[128 * i:128 * (i + 1)]
    return full


# revision 27
# speedup vs baseline: 1.0943x; 1.0943x over previous
"""HSTU block kernel for 8 TRN2 NeuronCores (nn_HSTU_66279935312625).

Sharding: 2 cores per batch (B=4). Core pair splits the 16 q-tiles
(128 rows each) causally balanced: g=0 owns tiles {15,13,11,9,6,4,2,0},
g=1 owns {14,12,10,8,7,5,3,1}, ordered descending-extent within two
512-row q-units so the moving q-stream width shrinks with k-block
index. Program k-extents per unit position: unit0 (16,14,12,10), unit1
(8,6,4,2); k-blocks iterate descending so PSUM AV accumulation uses one
bank-wide start (per-element has_written handles growing widths).
72 q128xk128 blocks per head vs 136 full-causal (optimum 68).

K/V are projected for a contiguous half of rows per core and exchanged
with the pair core via two DRAM AllGathers (K, then V), overlapped with
the Q/U projections; assembly DMAs ride the otherwise-idle gpsimd queue.

All matmuls bf16 (fp32 PSUM). Attention S and AV are emitted as
adjacent pairs on disjoint PE row/col groups (2-head packing) so the
16 32x32 sub-arrays run both heads concurrently; the emission is
software-pipelined (S-pair(kb) -> AV-pair(kb+1's prev) -> sigmoids) to
keep pairs adjacent in the PE queue. Biases fold into activation bias
APs / vector adds. LayerNorm stats are deferred after each unit's
attention (frees PSUM banks), gamma/beta apply via scalar activation
scale/bias APs, and the u0 tail (LN + out-proj, 2 PSUM banks) overlaps
u1 attention (6 banks). Final out = gated @ W_out + b_out + x.
"""
import numpy as np

import concourse.bacc as bacc
import concourse.tile as tile
from concourse import mybir
from concourse.bass_utils import run_bass_kernel_spmd
from concourse.tile_rust import add_dep_helper

F32 = mybir.dt.float32
F32R = mybir.dt.float32r
BF16 = mybir.dt.bfloat16
AF = mybir.ActivationFunctionType

B, L, D, H, HD = 4, 2048, 1024, 16, 64
OWN = 1024          # rows owned per core
UQ = 512            # rows per q-unit
EXT0 = (16, 14, 12, 10)   # program k-extent per q-tile position, unit 0
EXT1 = (8, 6, 4, 2)       # unit 1
SCALE = HD ** -0.5
LN_EPS = 1e-8
NCORES = 8

TILES0 = (15, 13, 11, 9, 6, 4, 2, 0)
TILES1 = (14, 12, 10, 8, 7, 5, 3, 1)

_CACHED = {}


def _build():
    nc = bacc.Bacc("TRN2", target_bir_lowering=False, debug=False)

    xkvT = nc.dram_tensor("xkvT", [D, OWN], BF16, kind="ExternalInput").ap()
    xqT = nc.dram_tensor("xqT", [D, OWN], BF16, kind="ExternalInput").ap()
    xq = nc.dram_tensor("xq", [OWN, D], F32, kind="ExternalInput").ap()
    wproj = nc.dram_tensor("wproj", [D, 4 * D], BF16, kind="ExternalInput").ap()
    wout = nc.dram_tensor("wout", [D, D], BF16, kind="ExternalInput").ap()
    cosk = nc.dram_tensor("cosk", [128, OWN], BF16, kind="ExternalInput").ap()
    sink = nc.dram_tensor("sink", [128, OWN], BF16, kind="ExternalInput").ap()
    cosq = nc.dram_tensor("cosq", [128, OWN], BF16, kind="ExternalInput").ap()
    sinq = nc.dram_tensor("sinq", [128, OWN], BF16, kind="ExternalInput").ap()
    p2 = nc.dram_tensor("p2", [128, 128], BF16, kind="ExternalInput").ap()
    maskT = nc.dram_tensor("maskT", [128, 2, 8, 128], BF16,
                           kind="ExternalInput").ap()
    bcolT = nc.dram_tensor("bcolT", [128, 32], F32, kind="ExternalInput").ap()
    gbT = nc.dram_tensor("gbT", [128, 16], F32, kind="ExternalInput").ap()
    vbias = nc.dram_tensor("vbias", [128, D], F32, kind="ExternalInput").ap()
    boutb = nc.dram_tensor("boutb", [128, D], F32, kind="ExternalInput").ap()
    ones128 = nc.dram_tensor("ones128", [128, 1], F32R, kind="ExternalInput").ap()
    onesrow = nc.dram_tensor("onesrow", [1, 128], F32R, kind="ExternalInput").ap()
    out = nc.dram_tensor("out", [OWN, D], F32, kind="ExternalOutput").ap()
    # pair-allgather bounce buffers
    kout = nc.dram_tensor("kout", [128, 8192], BF16, kind="Internal").ap()
    vout = nc.dram_tensor("vout", [128, 8192], BF16, kind="Internal").ap()
    kg = nc.dram_tensor("kg", [2, 128, 8192], BF16, kind="Internal").ap()
    vg = nc.dram_tensor("vg", [2, 128, 8192], BF16, kind="Internal").ap()

    wp3 = wproj.rearrange("(t ki) n -> ki t n", ki=128)   # [128, 8, 4096]
    wo3 = wout.rearrange("(t ki) n -> ki t n", ki=128)    # [128, 8, 1024]
    xkv3 = xkvT.rearrange("(t ki) n -> ki t n", ki=128)   # [128, 8, 1024]
    xq3 = xqT.rearrange("(t ki) n -> ki t n", ki=128)     # [128, 8, 1024]
    RG = [[0, 1], [2, 3], [4, 5], [6, 7]]

    with tile.TileContext(nc) as tc:
        with (
            tc.tile_pool(name="const", bufs=1) as cpool,
            tc.tile_pool(name="big", bufs=1) as big,
        ):
            p2sb = cpool.tile([128, 128], BF16)
            nc.sync.dma_start(p2sb[:], p2)
            bcsb = cpool.tile([128, 32], F32)
            nc.sync.dma_start(bcsb[:], bcolT)
            gbsb = cpool.tile([128, 16], F32)
            nc.sync.dma_start(gbsb[:], gbT)
            vbsb = cpool.tile([128, D], F32)
            nc.sync.dma_start(vbsb[:], vbias)
            bosb = cpool.tile([128, D], F32)
            nc.sync.dma_start(bosb[:], boutb)
            o128 = cpool.tile([128, 1], F32R)
            nc.sync.dma_start(o128[:], ones128)
            orow = cpool.tile([1, 128], F32R)
            nc.sync.dma_start(orow[:], onesrow)
            epsb = cpool.tile([1, 1], F32)
            nc.vector.memset(epsb[:], LN_EPS)

            krot = big.tile([128, 8, L], BF16)      # K_rot^T  [2-head col tiles]
            v16 = big.tile([128, 16, D], BF16)      # V natural [row tiles]
            qrot = big.tile([128, 8, OWN], BF16)    # Q_rot^T
            silu16 = big.tile([128, 8, OWN], BF16)  # silu(U)^T

            # ================= phase 1: K and V (own half) =================
            with (
                tc.tile_pool(name="rope", bufs=3) as rope,
                tc.tile_pool(name="ppj", bufs=4, space="PSUM") as ppj,
                tc.tile_pool(name="prt", bufs=2, space="PSUM") as prt,
            ):
                def rope_chain(psP, bias_sl, cos_sl, sin_sl, dst, rp, pp):
                    # dst(bf16) = (psP+b)*cos + rotate_half(psP+b)*sin
                    t16 = rp.tile([128, UQ], BF16, tag="t16")
                    nc.scalar.activation(t16[:], psP[:], AF.Identity,
                                         bias=bias_sl)
                    psR = pp.tile([128, UQ], F32, tag="psR")
                    nc.tensor.matmul(psR[:], p2sb[:], t16[:], start=True,
                                     stop=True)
                    tcos = rp.tile([128, UQ], BF16, tag="tcos")
                    nc.vector.tensor_mul(tcos[:], t16[:], cos_sl)
                    tsin = rp.tile([128, UQ], BF16, tag="tsin")
                    nc.vector.tensor_mul(tsin[:], psR[:], sin_sl)
                    nc.vector.tensor_add(dst, tcos[:], tsin[:])

                kv_scope = tc.tile_pool(name="ph1", bufs=1)
                ph1 = kv_scope.__enter__()
                wr_scope = tc.tile_pool(name="wring", bufs=3)
                wring = wr_scope.__enter__()
                st_scope = tc.tile_pool(name="stage", bufs=4)
                stage = st_scope.__enter__()

                xh = ph1.tile([128, 8, 1024], BF16, tag="xh")
                nc.sync.dma_start(xh[:], xkv3)
                cksb = ph1.tile([128, 1024], BF16, tag="cksb")
                nc.sync.dma_start(cksb[:], cosk)
                sksb = ph1.tile([128, 1024], BF16, tag="sksb")
                nc.sync.dma_start(sksb[:], sink)

                for ct in range(8):
                    c0 = 3 * D + 128 * ct
                    wk = wring.tile([128, 8, 128], BF16, tag="wk")
                    nc.sync.dma_start(wk[:], wp3[:, :, c0:c0 + 128])
                    for r in range(2):
                        ps = ppj.tile([128, UQ], F32, tag="ps")
                        for t in range(8):
                            nc.tensor.matmul(
                                ps[:], wk[:, t, :],
                                xh[:, t, r * UQ:(r + 1) * UQ],
                                start=(t == 0), stop=(t == 7))
                        off = r * UQ
                        kst = stage.tile([128, UQ], BF16, tag="kst")
                        rope_chain(ps, bcsb[:, 24 + ct:25 + ct],
                                   cksb[:, off:off + UQ],
                                   sksb[:, off:off + UQ],
                                   kst[:], rope, prt)
                        nc.sync.dma_start(
                            kout[:, 1024 * ct + off:1024 * ct + off + UQ],
                            kst[:])

                # K halves exchange; assembly DMAs on the idle gpsimd queue
                nc.gpsimd.collective_compute(
                    "AllGather", mybir.AluOpType.bypass, replica_groups=RG,
                    ins=[kout.opt()], outs=[kg.opt()])
                for gi in range(2):
                    kpart = kg[gi, :, :].rearrange("p (c n) -> p c n", c=8)
                    nc.gpsimd.dma_start(
                        krot[:, :, 1024 * gi:1024 * (gi + 1)], kpart)

                for vh in range(2):
                    v0 = D + UQ * vh
                    wvh = wring.tile([128, 8, UQ], BF16, tag="wv", bufs=1)
                    nc.sync.dma_start(wvh[:], wp3[:, :, v0:v0 + UQ])
                    for rv in range(8):
                        pv = ppj.tile([128, UQ], F32, tag="ps")
                        for t in range(8):
                            nc.tensor.matmul(
                                pv[:], xh[:, t, 128 * rv:128 * (rv + 1)],
                                wvh[:, t, :], start=(t == 0), stop=(t == 7))
                        vst = stage.tile([128, UQ], BF16, tag="vst")
                        nc.vector.tensor_add(
                            vst[:], pv[:], vbsb[:, UQ * vh:UQ * (vh + 1)])
                        nc.sync.dma_start(
                            vout[:, 1024 * rv + UQ * vh:
                                 1024 * rv + UQ * (vh + 1)],
                            vst[:])

                nc.gpsimd.collective_compute(
                    "AllGather", mybir.AluOpType.bypass, replica_groups=RG,
                    ins=[vout.opt()], outs=[vg.opt()])
                for gi in range(2):
                    vpart = vg[gi, :, :].rearrange("p (c n) -> p c n", c=8)
                    nc.gpsimd.dma_start(v16[:, 8 * gi:8 * (gi + 1), :], vpart)

                st_scope.__exit__(None, None, None)
                wr_scope.__exit__(None, None, None)
                kv_scope.__exit__(None, None, None)

                # ============= phase 1a: Q and U =============
                with (
                    tc.tile_pool(name="ph1a", bufs=1) as ph1a,
                    tc.tile_pool(name="wring2", bufs=2) as wring2,
                ):
                    xqsb = ph1a.tile([128, 8, OWN], BF16)
                    nc.sync.dma_start(xqsb[:], xq3)
                    cqsb = ph1a.tile([128, OWN], BF16)
                    nc.sync.dma_start(cqsb[:], cosq)
                    sqsb = ph1a.tile([128, OWN], BF16)
                    nc.sync.dma_start(sqsb[:], sinq)

                    for ct in range(8):
                        wu = wring2.tile([128, 8, 128], BF16, tag="wu")
                        nc.sync.dma_start(wu[:], wp3[:, :, 128 * ct:128 * (ct + 1)])
                        q0 = 2 * D + 128 * ct
                        wq = wring2.tile([128, 8, 128], BF16, tag="wq")
                        nc.sync.dma_start(wq[:], wp3[:, :, q0:q0 + 128])
                        for r in range(2):
                            sl = slice(r * UQ, (r + 1) * UQ)
                            psu = ppj.tile([128, UQ], F32, tag="ps")
                            for t in range(8):
                                nc.tensor.matmul(psu[:], wu[:, t, :],
                                                 xqsb[:, t, sl],
                                                 start=(t == 0), stop=(t == 7))
                            nc.scalar.activation(silu16[:, ct, sl], psu[:],
                                                 AF.Silu,
                                                 bias=bcsb[:, ct:ct + 1])
                            psq = ppj.tile([128, UQ], F32, tag="ps")
                            for t in range(8):
                                nc.tensor.matmul(psq[:], wq[:, t, :],
                                                 xqsb[:, t, sl],
                                                 start=(t == 0), stop=(t == 7))
                            rope_chain(psq, bcsb[:, 16 + ct:17 + ct],
                                       cqsb[:, sl], sqsb[:, sl],
                                       qrot[:, ct, sl], rope, prt)

            # ================= phase 2: attention + tail =================
            with (
                tc.tile_pool(name="ph2", bufs=1) as ph2,
                tc.tile_pool(name="mring", bufs=2) as mring,
                tc.tile_pool(name="aring", bufs=6) as aring,
                tc.tile_pool(name="sqring", bufs=2) as sqring,
                tc.tile_pool(name="gring", bufs=2) as gring,
                tc.tile_pool(name="oring", bufs=2) as oring,
                tc.tile_pool(name="psS", bufs=2, space="PSUM") as psSp,
                tc.tile_pool(name="psO", bufs=2, space="PSUM") as psOp,
            ):
                wosb = ph2.tile([128, 8, D], BF16)
                nc.sync.dma_start(wosb[:], wo3)
                attnT = ph2.tile([128, 8, UQ], F32R)
                gated = ph2.tile([128, 8, UQ], BF16)
                statr = ph2.tile([1, 4, UQ], F32R)

                for u in range(2):
                    EXT = EXT0 if u == 0 else EXT1
                    msb = mring.tile([128, 8, 128], BF16, tag="msb")
                    nc.sync.dma_start(msb[:], maskT[:, u, :, :])
                    for hp in range(8):
                        psO = psOp.tile([128, UQ], F32, tag="psO")
                        q0c = u * UQ
                        pend = []
                        # chain attention matmuls in emission order so the
                        # scheduler keeps row/col-group pairs adjacent in the
                        # PE queue (sub-array concurrency).
                        last_mm = [None]

                        def chain(mm):
                            if last_mm[0] is not None:
                                add_dep_helper(mm.ins, last_mm[0].ins,
                                               reason="pe-pair-order")
                            last_mm[0] = mm

                        def av_pair(pr):
                            kbp, wp, aABp = pr
                            st = kbp == EXT[0] - 1
                            sp = kbp == 0
                            chain(nc.tensor.matmul(
                                psO[0:64, 0:wp],
                                v16[:, kbp, 128 * hp:128 * hp + 64],
                                aABp[:, 0, 0:wp], start=st, stop=sp,
                                tile_position=(0, 0)))
                            chain(nc.tensor.matmul(
                                psO[64:128, 0:wp],
                                v16[:, kbp, 128 * hp + 64:128 * (hp + 1)],
                                aABp[:, 1, 0:wp], start=st, stop=sp,
                                tile_position=(0, 64)))

                        for kb in range(EXT[0] - 1, -1, -1):
                            nP = sum(1 for e in EXT if e > kb)
                            w = 128 * nP
                            psAB = psSp.tile([128, 2, UQ], F32, tag="psS")
                            kbs = slice(128 * kb, 128 * (kb + 1))
                            chain(nc.tensor.matmul(
                                psAB[:, 0, 0:w], krot[0:64, hp, kbs],
                                qrot[0:64, hp, q0c:q0c + w],
                                start=True, stop=True))
                            chain(nc.tensor.matmul(
                                psAB[:, 1, 0:w], krot[64:128, hp, kbs],
                                qrot[64:128, hp, q0c:q0c + w],
                                start=True, stop=True))
                            if len(pend) >= 2:
                                av_pair(pend.pop(0))
                            aAB = aring.tile([128, 2, UQ], BF16, tag="aAB")
                            nc.scalar.activation(aAB[:, :, 0:w],
                                                 psAB[:, :, 0:w],
                                                 AF.Sigmoid, scale=SCALE)
                            pl = nP - 1
                            if kb >= EXT[pl] - 2:
                                m = 2 * pl + (1 if kb == EXT[pl] - 2 else 0)
                                nc.vector.tensor_mul(
                                    aAB[:, 0, w - 128:w],
                                    aAB[:, 0, w - 128:w], msb[:, m, :])
                                nc.vector.tensor_mul(
                                    aAB[:, 1, w - 128:w],
                                    aAB[:, 1, w - 128:w], msb[:, m, :])
                            pend.append((kb, w, aAB))
                        for pr in pend:
                            av_pair(pr)
                        nc.vector.tensor_copy(attnT[:, hp, :], psO[:])

                    # ---- deferred LN stats ----
                    with tc.tile_pool(name=f"psT{u}", bufs=2,
                                      space="PSUM") as psTp:
                        psSum = psTp.tile([1, UQ], F32, tag="st")
                        psSq = psTp.tile([1, UQ], F32, tag="st")
                        for c in range(8):
                            sq = sqring.tile([128, UQ], F32R, tag="sq")
                            nc.vector.tensor_mul(sq[:], attnT[:, c, :],
                                                 attnT[:, c, :])
                            nc.tensor.matmul(psSum[:], o128[:],
                                             attnT[:, c, :],
                                             start=(c == 0), stop=(c == 7))
                            nc.tensor.matmul(psSq[:], o128[:], sq[:],
                                             start=(c == 0), stop=(c == 7))
                        mu = statr[0:1, 0, :]
                        nc.vector.tensor_scalar_mul(mu, psSum[:], 1.0 / D)
                        m2 = statr[0:1, 1, :]
                        nc.vector.tensor_scalar_mul(m2, psSq[:], 1.0 / D)
                        musq = statr[0:1, 2, :]
                        nc.vector.tensor_mul(musq, mu, mu)
                        varr = statr[0:1, 1, :]
                        nc.vector.tensor_sub(varr, m2, musq)
                        rstd = statr[0:1, 3, :]
                        nc.scalar.activation(rstd, varr, AF.Sqrt, bias=epsb[:])
                        with nc.allow_low_precision("f32r rstd for matmul"):
                            nc.vector.reciprocal(rstd, rstd)
                        nmr = statr[0:1, 2, :]
                        nc.vector.tensor_mul(nmr, mu, rstd)
                        nc.vector.tensor_scalar_mul(nmr, nmr, -1.0)

                    # ---- LN apply + gating ----
                    with tc.tile_pool(name=f"psG{u}", bufs=2,
                                      space="PSUM") as psGp:
                        psRB = psGp.tile([128, UQ], F32, tag="bc")
                        nc.tensor.matmul(psRB[:], orow[:], statr[0:1, 3, :],
                                         start=True, stop=True)
                        psNB = psGp.tile([128, UQ], F32, tag="bc")
                        nc.tensor.matmul(psNB[:], orow[:], statr[0:1, 2, :],
                                         start=True, stop=True)
                        for c in range(8):
                            g1 = gring.tile([128, UQ], F32, tag="g1")
                            nc.vector.tensor_mul(g1[:], attnT[:, c, :],
                                                 psRB[:])
                            g2 = gring.tile([128, UQ], F32, tag="g2")
                            nc.vector.tensor_add(g2[:], g1[:], psNB[:])
                            g3 = gring.tile([128, UQ], F32, tag="g3")
                            nc.scalar.activation(g3[:], g2[:], AF.Identity,
                                                 scale=gbsb[:, c:c + 1],
                                                 bias=gbsb[:, 8 + c:9 + c])
                            nc.vector.tensor_mul(
                                gated[:, c, :], g3[:],
                                silu16[:, c, u * UQ:(u + 1) * UQ])

                    # ---- out projection (2 PSUM banks) ----
                    with tc.tile_pool(name=f"psP{u}", bufs=2,
                                      space="PSUM") as psPp:
                        for rw in range(4):
                            r0 = u * UQ + 128 * rw
                            xqn = oring.tile([128, D], F32, tag="xqn")
                            nc.sync.dma_start(xqn[:], xq[r0:r0 + 128, :])
                            pbs = [psPp.tile([128, UQ], F32, tag="po",
                                             name=f"po{u}_{rw}_{i}")
                                   for i in range(2)]
                            for c in range(8):
                                st = gated[:, c, 128 * rw:128 * (rw + 1)]
                                for oh in range(2):
                                    nc.tensor.matmul(
                                        pbs[oh][:], st,
                                        wosb[:, c, UQ * oh:UQ * (oh + 1)],
                                        start=(c == 0), stop=(c == 7))
                            for oh in range(2):
                                ohs = slice(UQ * oh, UQ * (oh + 1))
                                osb = oring.tile([128, UQ], F32, tag="osb")
                                nc.vector.tensor_add(osb[:], pbs[oh][:],
                                                     xqn[:, ohs])
                                osb2 = oring.tile([128, UQ], F32, tag="osb2")
                                nc.gpsimd.tensor_add(osb2[:], osb[:],
                                                     bosb[:, ohs])
                                nc.sync.dma_start(out[r0:r0 + 128, ohs],
                                                  osb2[:])
    nc.finalize()
    return nc


def _host_prep(x, attn_mask, W_proj, b_proj, ln_gamma, ln_beta, W_out, b_out):
    """Build the 8 per-core input maps."""
    import ml_dtypes
    bf16 = ml_dtypes.bfloat16

    x = np.asarray(x, dtype=np.float32)
    attn_mask = np.asarray(attn_mask)
    W_proj = np.ascontiguousarray(np.asarray(W_proj, dtype=np.float32))
    W_out = np.ascontiguousarray(np.asarray(W_out, dtype=np.float32))
    b_proj = np.asarray(b_proj, dtype=np.float32)
    b_out = np.asarray(b_out, dtype=np.float32)
    ln_gamma = np.asarray(ln_gamma, dtype=np.float32)
    ln_beta = np.asarray(ln_beta, dtype=np.float32)

    inv = 1.0 / (10000.0 ** (np.arange(0, HD, 2, dtype=np.float64) / HD))
    ang = np.outer(inv, np.arange(L, dtype=np.float64))       # [32, L]
    c64 = np.concatenate([np.cos(ang), np.cos(ang)], 0)
    s64 = np.concatenate([np.sin(ang), np.sin(ang)], 0)
    cosk = np.concatenate([c64, c64], 0).astype(np.float32)   # [128, L]
    sink = np.concatenate([s64, s64], 0).astype(np.float32)

    p2 = np.zeros((128, 128), dtype=np.float32)
    for base in (0, 64):
        for m in range(32):
            p2[base + m + 32, base + m] = -1.0
        for m in range(32, 64):
            p2[base + m - 32, base + m] = 1.0

    # per-partition column biases [128, 32]: sections U,V,Q,K x 8 chunks
    bcolT = np.empty((128, 32), dtype=np.float32)
    for s in range(4):
        for c in range(8):
            bcolT[:, 8 * s + c] = b_proj[s * D + 128 * c:s * D + 128 * (c + 1)]
    gbT = np.empty((128, 16), dtype=np.float32)
    for c in range(8):
        gbT[:, c] = ln_gamma[128 * c:128 * (c + 1)]
        gbT[:, 8 + c] = ln_beta[128 * c:128 * (c + 1)]
    vbias = np.broadcast_to(b_proj[D:2 * D], (128, D))
    boutb = np.broadcast_to(b_out, (128, D))

    shared = dict(
        wproj=W_proj.astype(bf16), wout=W_out.astype(bf16),
        p2=p2.astype(bf16),
        bcolT=bcolT, gbT=gbT,
        vbias=np.ascontiguousarray(vbias),
        boutb=np.ascontiguousarray(boutb),
        ones128=np.ones((128, 1), np.float32),
        onesrow=np.ones((1, 128), np.float32),
    )

    exts = (EXT0, EXT1)
    in_maps = []
    for cid in range(NCORES):
        b, g = divmod(cid, 2)
        tiles = TILES0 if g == 0 else TILES1
        own = np.concatenate([np.arange(128 * t, 128 * (t + 1))
                              for t in tiles])
        xb = x[b]
        xqc = np.ascontiguousarray(xb[own])
        half = slice(OWN * g, OWN * (g + 1))
        m = dict(shared)
        m["xkvT"] = np.ascontiguousarray(xb[half].T).astype(bf16)
        m["cosk"] = np.ascontiguousarray(cosk[:, half]).astype(bf16)
        m["sink"] = np.ascontiguousarray(sink[:, half]).astype(bf16)
        m["xqT"] = np.ascontiguousarray(xqc.T).astype(bf16)
        m["xq"] = xqc
        m["cosq"] = np.ascontiguousarray(cosk[:, own]).astype(bf16)
        m["sinq"] = np.ascontiguousarray(sink[:, own]).astype(bf16)
        am = attn_mask[b]
        mk = np.zeros((128, 2, 8, 128), dtype=np.float32)
        for uu in range(2):
            for p in range(4):
                t = tiles[4 * uu + p]
                qrows = slice(128 * t, 128 * (t + 1))
                for w01 in range(2):
                    kb = exts[uu][p] - 1 - w01
                    mk[:, uu, 2 * p + w01, :] = \
                        am[qrows, 128 * kb:128 * (kb + 1)].T
        m["maskT"] = mk.astype(bf16)
        in_maps.append(m)
    return in_maps


def kernel(**inputs):
    if "nc" not in _CACHED:
        _CACHED["nc"] = _build()
    nc = _CACHED["nc"]
    in_maps = _host_prep(**inputs)
    res = run_bass_kernel_spmd(nc, in_maps, list(range(NCORES)))
    globals()["_LAST_RESULTS"] = res
    full = np.empty((B, L, D), dtype=np.float32)
    for cid in range(NCORES):
        b, g = divmod(cid, 2)
        tiles = TILES0 if g == 0 else TILES1
        o = res.results[cid]["out"]
        for i, t in enumerate(tiles):
            full[b, 128 * t:128 * (t + 1)] = o[128 * i:128 * (i + 1)]
    return full


# revision 36
# speedup vs baseline: 1.1536x; 1.0542x over previous
"""HSTU block kernel for 8 TRN2 NeuronCores (nn_HSTU_66279935312625).

Sharding: 2 cores per batch (B=4). Core pair splits the 16 q-tiles
(128 rows each) causally balanced: g=0 owns tiles {15,13,11,9,6,4,2,0},
g=1 owns {14,12,10,8,7,5,3,1}, ordered descending-extent within two
512-row q-units so the moving q-stream width shrinks with k-block
index. Program k-extents per unit position: unit0 (16,14,12,10), unit1
(8,6,4,2); k-blocks iterate descending so PSUM AV accumulation uses one
bank-wide start (per-element has_written handles growing widths).
72 q128xk128 blocks per head vs 136 full-causal (optimum 68).

K/V are projected for a contiguous half of rows per core and exchanged
with the pair core via two DRAM AllGathers (K, then V), overlapped with
the Q/U projections; assembly DMAs ride the otherwise-idle gpsimd queue.

All matmuls bf16 (fp32 PSUM). Attention S and AV are emitted as
adjacent pairs on disjoint PE row/col groups (2-head packing) so the
16 32x32 sub-arrays run both heads concurrently; the emission is
software-pipelined (S-pair(kb) -> AV-pair(kb+1's prev) -> sigmoids) to
keep pairs adjacent in the PE queue. Biases fold into activation bias
APs / vector adds. LayerNorm stats are deferred after each unit's
attention (frees PSUM banks), gamma/beta apply via scalar activation
scale/bias APs, and the u0 tail (LN + out-proj, 2 PSUM banks) overlaps
u1 attention (6 banks). Final out = gated @ W_out + b_out + x.
"""
import numpy as np

import concourse.bacc as bacc
import concourse.tile as tile
from concourse import mybir
from concourse.bass_utils import run_bass_kernel_spmd
from concourse.tile_rust import add_dep_helper

F32 = mybir.dt.float32
F32R = mybir.dt.float32r
BF16 = mybir.dt.bfloat16
AF = mybir.ActivationFunctionType

B, L, D, H, HD = 4, 2048, 1024, 16, 64
OWN = 1024          # rows owned per core
UQ = 512            # rows per q-unit
EXT0 = (16, 14, 12, 10)   # program k-extent per q-tile position, unit 0
EXT1 = (8, 6, 4, 2)       # unit 1
SCALE = HD ** -0.5
LN_EPS = 1e-8
NCORES = 8

TILES0 = (15, 13, 11, 9, 6, 4, 2, 0)
TILES1 = (14, 12, 10, 8, 7, 5, 3, 1)

_CACHED = {}


def _build():
    nc = bacc.Bacc("TRN2", target_bir_lowering=False, debug=False)

    xkvT = nc.dram_tensor("xkvT", [D, OWN], BF16, kind="ExternalInput").ap()
    xqT = nc.dram_tensor("xqT", [D, OWN], BF16, kind="ExternalInput").ap()
    xq = nc.dram_tensor("xq", [OWN, D], F32, kind="ExternalInput").ap()
    # weights pre-laid-out per DMA tile: [128, chunk, t, cols] contiguous
    wur = nc.dram_tensor("wur", [128, 8, 8, 128], BF16, kind="ExternalInput").ap()
    wvr = nc.dram_tensor("wvr", [128, 2, 8, 512], BF16, kind="ExternalInput").ap()
    wqr = nc.dram_tensor("wqr", [128, 8, 8, 128], BF16, kind="ExternalInput").ap()
    wkr = nc.dram_tensor("wkr", [128, 8, 8, 128], BF16, kind="ExternalInput").ap()
    wout = nc.dram_tensor("wout", [D, D], BF16, kind="ExternalInput").ap()
    cosk = nc.dram_tensor("cosk", [128, OWN], BF16, kind="ExternalInput").ap()
    sink = nc.dram_tensor("sink", [128, OWN], BF16, kind="ExternalInput").ap()
    cosq = nc.dram_tensor("cosq", [128, OWN], BF16, kind="ExternalInput").ap()
    sinq = nc.dram_tensor("sinq", [128, OWN], BF16, kind="ExternalInput").ap()
    p2 = nc.dram_tensor("p2", [128, 128], BF16, kind="ExternalInput").ap()
    maskT = nc.dram_tensor("maskT", [2, 128, 8, 128], BF16,
                           kind="ExternalInput").ap()
    bcolT = nc.dram_tensor("bcolT", [128, 32], F32, kind="ExternalInput").ap()
    gbT = nc.dram_tensor("gbT", [128, 16], F32, kind="ExternalInput").ap()
    vbias = nc.dram_tensor("vbias", [128, D], F32, kind="ExternalInput").ap()
    boutb = nc.dram_tensor("boutb", [128, D], F32, kind="ExternalInput").ap()
    ones128 = nc.dram_tensor("ones128", [128, 1], F32R, kind="ExternalInput").ap()
    onesrow = nc.dram_tensor("onesrow", [1, 128], F32R, kind="ExternalInput").ap()
    out = nc.dram_tensor("out", [OWN, D], F32, kind="ExternalOutput").ap()
    # pair-allgather bounce buffers
    kout = nc.dram_tensor("kout", [128, 8192], BF16, kind="Internal").ap()
    vout = nc.dram_tensor("vout", [128, 8192], BF16, kind="Internal").ap()
    kg = nc.dram_tensor("kg", [2, 128, 8192], BF16, kind="Internal").ap()
    vg = nc.dram_tensor("vg", [2, 128, 8192], BF16, kind="Internal").ap()

    wo3 = wout.rearrange("(t ki) n -> ki t n", ki=128)    # [128, 8, 1024]
    xkv3 = xkvT.rearrange("(t ki) n -> ki t n", ki=128)   # [128, 8, 1024]
    xq3 = xqT.rearrange("(t ki) n -> ki t n", ki=128)     # [128, 8, 1024]
    RG = [[0, 1], [2, 3], [4, 5], [6, 7]]

    with tile.TileContext(nc) as tc:
        with (
            tc.tile_pool(name="const", bufs=1) as cpool,
            tc.tile_pool(name="big", bufs=1) as big,
        ):
            p2sb = cpool.tile([128, 128], BF16)
            nc.sync.dma_start(p2sb[:], p2)
            bcsb = cpool.tile([128, 32], F32)
            nc.sync.dma_start(bcsb[:], bcolT)
            gbsb = cpool.tile([128, 16], F32)
            nc.sync.dma_start(gbsb[:], gbT)
            vbsb = cpool.tile([128, D], F32)
            nc.sync.dma_start(vbsb[:], vbias)
            bosb = cpool.tile([128, D], F32)
            nc.sync.dma_start(bosb[:], boutb)
            o128 = cpool.tile([128, 1], F32R)
            nc.sync.dma_start(o128[:], ones128)
            orow = cpool.tile([1, 128], F32R)
            nc.sync.dma_start(orow[:], onesrow)
            epsb = cpool.tile([1, 1], F32)
            nc.vector.memset(epsb[:], LN_EPS)

            krot = big.tile([128, 8, L], BF16)      # K_rot^T  [2-head col tiles]
            v16 = big.tile([128, 16, D], BF16)      # V natural [row tiles]
            qrot = big.tile([128, 8, OWN], BF16)    # Q_rot^T
            silu16 = big.tile([128, 8, OWN], BF16)  # silu(U)^T

            # ================= phase 1: K and V (own half) =================
            with (
                tc.tile_pool(name="rope", bufs=3) as rope,
                tc.tile_pool(name="ppj", bufs=4, space="PSUM") as ppj,
                tc.tile_pool(name="prt", bufs=2, space="PSUM") as prt,
            ):
                def rope_chain(psP, bias_sl, cos_sl, sin_sl, dst, rp, pp):
                    # dst(bf16) = (psP+b)*cos + rotate_half(psP+b)*sin
                    t16 = rp.tile([128, UQ], BF16, tag="t16")
                    nc.scalar.activation(t16[:], psP[:], AF.Identity,
                                         bias=bias_sl)
                    psR = pp.tile([128, UQ], F32, tag="psR")
                    nc.tensor.matmul(psR[:], p2sb[:], t16[:], start=True,
                                     stop=True)
                    tcos = rp.tile([128, UQ], BF16, tag="tcos")
                    nc.vector.tensor_mul(tcos[:], t16[:], cos_sl)
                    tsin = rp.tile([128, UQ], BF16, tag="tsin")
                    nc.vector.tensor_mul(tsin[:], psR[:], sin_sl)
                    nc.vector.tensor_add(dst, tcos[:], tsin[:])

                kv_scope = tc.tile_pool(name="ph1", bufs=1)
                ph1 = kv_scope.__enter__()
                wr_scope = tc.tile_pool(name="wring", bufs=3)
                wring = wr_scope.__enter__()
                st_scope = tc.tile_pool(name="stage", bufs=4)
                stage = st_scope.__enter__()

                xh = ph1.tile([128, 8, 1024], BF16, tag="xh")
                nc.sync.dma_start(xh[:], xkv3)
                cksb = ph1.tile([128, 1024], BF16, tag="cksb")
                nc.sync.dma_start(cksb[:], cosk)
                sksb = ph1.tile([128, 1024], BF16, tag="sksb")
                nc.sync.dma_start(sksb[:], sink)

                for ct in range(8):
                    wk = wring.tile([128, 8, 128], BF16, tag="wk")
                    nc.sync.dma_start(wk[:], wkr[:, ct, :, :])
                    for r in range(2):
                        ps = ppj.tile([128, UQ], F32, tag="ps")
                        for t in range(8):
                            nc.tensor.matmul(
                                ps[:], wk[:, t, :],
                                xh[:, t, r * UQ:(r + 1) * UQ],
                                start=(t == 0), stop=(t == 7))
                        off = r * UQ
                        kst = stage.tile([128, UQ], BF16, tag="kst")
                        rope_chain(ps, bcsb[:, 24 + ct:25 + ct],
                                   cksb[:, off:off + UQ],
                                   sksb[:, off:off + UQ],
                                   kst[:], rope, prt)
                        nc.sync.dma_start(
                            kout[:, 1024 * ct + off:1024 * ct + off + UQ],
                            kst[:])

                # K halves exchange; assembly DMAs on the idle gpsimd queue
                nc.gpsimd.collective_compute(
                    "AllGather", mybir.AluOpType.bypass, replica_groups=RG,
                    ins=[kout.opt()], outs=[kg.opt()])
                for gi in range(2):
                    kpart = kg[gi, :, :].rearrange("p (c n) -> p c n", c=8)
                    nc.gpsimd.dma_start(
                        krot[:, :, 1024 * gi:1024 * (gi + 1)], kpart)

                for vh in range(2):
                    wvh = wring.tile([128, 8, UQ], BF16, tag="wv", bufs=1)
                    nc.sync.dma_start(wvh[:], wvr[:, vh, :, :])
                    for rv in range(8):
                        pv = ppj.tile([128, UQ], F32, tag="ps")
                        for t in range(8):
                            nc.tensor.matmul(
                                pv[:], xh[:, t, 128 * rv:128 * (rv + 1)],
                                wvh[:, t, :], start=(t == 0), stop=(t == 7))
                        vst = stage.tile([128, UQ], BF16, tag="vst")
                        nc.vector.tensor_add(
                            vst[:], pv[:], vbsb[:, UQ * vh:UQ * (vh + 1)])
                        nc.sync.dma_start(
                            vout[:, 1024 * rv + UQ * vh:
                                 1024 * rv + UQ * (vh + 1)],
                            vst[:])

                nc.gpsimd.collective_compute(
                    "AllGather", mybir.AluOpType.bypass, replica_groups=RG,
                    ins=[vout.opt()], outs=[vg.opt()])
                for gi in range(2):
                    vpart = vg[gi, :, :].rearrange("p (c n) -> p c n", c=8)
                    nc.gpsimd.dma_start(v16[:, 8 * gi:8 * (gi + 1), :], vpart)

                st_scope.__exit__(None, None, None)
                wr_scope.__exit__(None, None, None)
                kv_scope.__exit__(None, None, None)

                # ============= phase 1a: Q and U =============
                with (
                    tc.tile_pool(name="ph1a", bufs=1) as ph1a,
                    tc.tile_pool(name="wring2", bufs=2) as wring2,
                ):
                    xqsb = ph1a.tile([128, 8, OWN], BF16)
                    nc.sync.dma_start(xqsb[:], xq3)
                    cqsb = ph1a.tile([128, OWN], BF16)
                    nc.sync.dma_start(cqsb[:], cosq)
                    sqsb = ph1a.tile([128, OWN], BF16)
                    nc.sync.dma_start(sqsb[:], sinq)

                    for ct in range(8):
                        wu = wring2.tile([128, 8, 128], BF16, tag="wu")
                        nc.sync.dma_start(wu[:], wur[:, ct, :, :])
                        wq = wring2.tile([128, 8, 128], BF16, tag="wq")
                        nc.sync.dma_start(wq[:], wqr[:, ct, :, :])
                        for r in range(2):
                            sl = slice(r * UQ, (r + 1) * UQ)
                            psu = ppj.tile([128, UQ], F32, tag="ps")
                            for t in range(8):
                                nc.tensor.matmul(psu[:], wu[:, t, :],
                                                 xqsb[:, t, sl],
                                                 start=(t == 0), stop=(t == 7))
                            nc.scalar.activation(silu16[:, ct, sl], psu[:],
                                                 AF.Silu,
                                                 bias=bcsb[:, ct:ct + 1])
                            psq = ppj.tile([128, UQ], F32, tag="ps")
                            for t in range(8):
                                nc.tensor.matmul(psq[:], wq[:, t, :],
                                                 xqsb[:, t, sl],
                                                 start=(t == 0), stop=(t == 7))
                            rope_chain(psq, bcsb[:, 16 + ct:17 + ct],
                                       cqsb[:, sl], sqsb[:, sl],
                                       qrot[:, ct, sl], rope, prt)

            # ================= phase 2: attention + tail =================
            with (
                tc.tile_pool(name="ph2", bufs=1) as ph2,
                tc.tile_pool(name="mring", bufs=2) as mring,
                tc.tile_pool(name="aring", bufs=6) as aring,
                tc.tile_pool(name="sqring", bufs=2) as sqring,
                tc.tile_pool(name="gring", bufs=2) as gring,
                tc.tile_pool(name="oring", bufs=2) as oring,
                tc.tile_pool(name="psS", bufs=2, space="PSUM") as psSp,
                tc.tile_pool(name="psO", bufs=2, space="PSUM") as psOp,
            ):
                wosb = ph2.tile([128, 8, D], BF16)
                nc.sync.dma_start(wosb[:], wo3)
                attnT = ph2.tile([128, 8, UQ], F32R)
                gated = ph2.tile([128, 8, UQ], BF16)
                statr = ph2.tile([1, 4, UQ], F32R)

                for u in range(2):
                    EXT = EXT0 if u == 0 else EXT1
                    msb = mring.tile([128, 8, 128], BF16, tag="msb")
                    nc.sync.dma_start(msb[:], maskT[u, :, :, :])
                    for hp in range(8):
                        psO = psOp.tile([128, UQ], F32, tag="psO")
                        q0c = u * UQ
                        pend = []
                        # chain attention matmuls in emission order so the
                        # scheduler keeps row/col-group pairs adjacent in the
                        # PE queue (sub-array concurrency).
                        last_mm = [None]

                        def chain(mm):
                            if last_mm[0] is not None:
                                add_dep_helper(mm.ins, last_mm[0].ins,
                                               reason="pe-pair-order")
                            last_mm[0] = mm

                        def av_pair(pr):
                            kbp, wp, aABp = pr
                            st = kbp == EXT[0] - 1
                            sp = kbp == 0
                            chain(nc.tensor.matmul(
                                psO[0:64, 0:wp],
                                v16[:, kbp, 128 * hp:128 * hp + 64],
                                aABp[:, 0, 0:wp], start=st, stop=sp,
                                tile_position=(0, 0)))
                            chain(nc.tensor.matmul(
                                psO[64:128, 0:wp],
                                v16[:, kbp, 128 * hp + 64:128 * (hp + 1)],
                                aABp[:, 1, 0:wp], start=st, stop=sp,
                                tile_position=(0, 64)))

                        for kb in range(EXT[0] - 1, -1, -1):
                            nP = sum(1 for e in EXT if e > kb)
                            w = 128 * nP
                            psAB = psSp.tile([128, 2, UQ], F32, tag="psS")
                            kbs = slice(128 * kb, 128 * (kb + 1))
                            chain(nc.tensor.matmul(
                                psAB[:, 0, 0:w], krot[0:64, hp, kbs],
                                qrot[0:64, hp, q0c:q0c + w],
                                start=True, stop=True))
                            chain(nc.tensor.matmul(
                                psAB[:, 1, 0:w], krot[64:128, hp, kbs],
                                qrot[64:128, hp, q0c:q0c + w],
                                start=True, stop=True))
                            if len(pend) >= 2:
                                av_pair(pend.pop(0))
                            aAB = aring.tile([128, 2, UQ], BF16, tag="aAB")
                            nc.scalar.activation(aAB[:, :, 0:w],
                                                 psAB[:, :, 0:w],
                                                 AF.Sigmoid, scale=SCALE)
                            pl = nP - 1
                            if kb >= EXT[pl] - 2:
                                m = 2 * pl + (1 if kb == EXT[pl] - 2 else 0)
                                nc.vector.tensor_mul(
                                    aAB[:, 0, w - 128:w],
                                    aAB[:, 0, w - 128:w], msb[:, m, :])
                                nc.vector.tensor_mul(
                                    aAB[:, 1, w - 128:w],
                                    aAB[:, 1, w - 128:w], msb[:, m, :])
                            pend.append((kb, w, aAB))
                        for pr in pend:
                            av_pair(pr)
                        nc.vector.tensor_copy(attnT[:, hp, :], psO[:])

                    # ---- deferred LN stats ----
                    with tc.tile_pool(name=f"psT{u}", bufs=2,
                                      space="PSUM") as psTp:
                        psSum = psTp.tile([1, UQ], F32, tag="st")
                        psSq = psTp.tile([1, UQ], F32, tag="st")
                        for c in range(8):
                            sq = sqring.tile([128, UQ], F32R, tag="sq")
                            nc.vector.tensor_mul(sq[:], attnT[:, c, :],
                                                 attnT[:, c, :])
                            nc.tensor.matmul(psSum[:], o128[:],
                                             attnT[:, c, :],
                                             start=(c == 0), stop=(c == 7))
                            nc.tensor.matmul(psSq[:], o128[:], sq[:],
                                             start=(c == 0), stop=(c == 7))
                        mu = statr[0:1, 0, :]
                        nc.vector.tensor_scalar_mul(mu, psSum[:], 1.0 / D)
                        m2 = statr[0:1, 1, :]
                        nc.vector.tensor_scalar_mul(m2, psSq[:], 1.0 / D)
                        musq = statr[0:1, 2, :]
                        nc.vector.tensor_mul(musq, mu, mu)
                        varr = statr[0:1, 1, :]
                        nc.vector.tensor_sub(varr, m2, musq)
                        rstd = statr[0:1, 3, :]
                        nc.scalar.activation(rstd, varr, AF.Sqrt, bias=epsb[:])
                        with nc.allow_low_precision("f32r rstd for matmul"):
                            nc.vector.reciprocal(rstd, rstd)
                        nmr = statr[0:1, 2, :]
                        nc.vector.tensor_mul(nmr, mu, rstd)
                        nc.vector.tensor_scalar_mul(nmr, nmr, -1.0)

                    # ---- LN apply + gating ----
                    with tc.tile_pool(name=f"psG{u}", bufs=2,
                                      space="PSUM") as psGp:
                        psRB = psGp.tile([128, UQ], F32, tag="bc")
                        nc.tensor.matmul(psRB[:], orow[:], statr[0:1, 3, :],
                                         start=True, stop=True)
                        psNB = psGp.tile([128, UQ], F32, tag="bc")
                        nc.tensor.matmul(psNB[:], orow[:], statr[0:1, 2, :],
                                         start=True, stop=True)
                        for c in range(8):
                            g1 = gring.tile([128, UQ], F32, tag="g1")
                            nc.vector.tensor_mul(g1[:], attnT[:, c, :],
                                                 psRB[:])
                            g2 = gring.tile([128, UQ], F32, tag="g2")
                            nc.vector.tensor_add(g2[:], g1[:], psNB[:])
                            g3 = gring.tile([128, UQ], F32, tag="g3")
                            nc.scalar.activation(g3[:], g2[:], AF.Identity,
                                                 scale=gbsb[:, c:c + 1],
                                                 bias=gbsb[:, 8 + c:9 + c])
                            nc.vector.tensor_mul(
                                gated[:, c, :], g3[:],
                                silu16[:, c, u * UQ:(u + 1) * UQ])

                    # ---- out projection (2 PSUM banks) ----
                    with tc.tile_pool(name=f"psP{u}", bufs=2,
                                      space="PSUM") as psPp:
                        for rw in range(4):
                            r0 = u * UQ + 128 * rw
                            xqn = oring.tile([128, D], F32, tag="xqn")
                            nc.sync.dma_start(xqn[:], xq[r0:r0 + 128, :])
                            pbs = [psPp.tile([128, UQ], F32, tag="po",
                                             name=f"po{u}_{rw}_{i}")
                                   for i in range(2)]
                            for c in range(8):
                                st = gated[:, c, 128 * rw:128 * (rw + 1)]
                                for oh in range(2):
                                    nc.tensor.matmul(
                                        pbs[oh][:], st,
                                        wosb[:, c, UQ * oh:UQ * (oh + 1)],
                                        start=(c == 0), stop=(c == 7))
                            for oh in range(2):
                                ohs = slice(UQ * oh, UQ * (oh + 1))
                                osb = oring.tile([128, UQ], F32, tag="osb")
                                nc.vector.tensor_add(osb[:], pbs[oh][:],
                                                     xqn[:, ohs])
                                osb2 = oring.tile([128, UQ], F32, tag="osb2")
                                nc.gpsimd.tensor_add(osb2[:], osb[:],
                                                     bosb[:, ohs])
                                nc.sync.dma_start(out[r0:r0 + 128, ohs],
                                                  osb2[:])
    nc.finalize()
    return nc


def _host_prep(x, attn_mask, W_proj, b_proj, ln_gamma, ln_beta, W_out, b_out):
    """Build the 8 per-core input maps."""
    import ml_dtypes
    bf16 = ml_dtypes.bfloat16

    x = np.asarray(x, dtype=np.float32)
    attn_mask = np.asarray(attn_mask)
    W_proj = np.ascontiguousarray(np.asarray(W_proj, dtype=np.float32))
    W_out = np.ascontiguousarray(np.asarray(W_out, dtype=np.float32))
    b_proj = np.asarray(b_proj, dtype=np.float32)
    b_out = np.asarray(b_out, dtype=np.float32)
    ln_gamma = np.asarray(ln_gamma, dtype=np.float32)
    ln_beta = np.asarray(ln_beta, dtype=np.float32)

    inv = 1.0 / (10000.0 ** (np.arange(0, HD, 2, dtype=np.float64) / HD))
    ang = np.outer(inv, np.arange(L, dtype=np.float64))       # [32, L]
    c64 = np.concatenate([np.cos(ang), np.cos(ang)], 0)
    s64 = np.concatenate([np.sin(ang), np.sin(ang)], 0)
    cosk = np.concatenate([c64, c64], 0).astype(np.float32)   # [128, L]
    sink = np.concatenate([s64, s64], 0).astype(np.float32)

    p2 = np.zeros((128, 128), dtype=np.float32)
    for base in (0, 64):
        for m in range(32):
            p2[base + m + 32, base + m] = -1.0
        for m in range(32, 64):
            p2[base + m - 32, base + m] = 1.0

    # per-partition column biases [128, 32]: sections U,V,Q,K x 8 chunks
    bcolT = np.empty((128, 32), dtype=np.float32)
    for s in range(4):
        for c in range(8):
            bcolT[:, 8 * s + c] = b_proj[s * D + 128 * c:s * D + 128 * (c + 1)]
    gbT = np.empty((128, 16), dtype=np.float32)
    for c in range(8):
        gbT[:, c] = ln_gamma[128 * c:128 * (c + 1)]
        gbT[:, 8 + c] = ln_beta[128 * c:128 * (c + 1)]
    vbias = np.broadcast_to(b_proj[D:2 * D], (128, D))
    boutb = np.broadcast_to(b_out, (128, D))

    # weights pre-laid-out per DMA tile: [128 ki, chunk, t, cols]
    wp4 = W_proj.astype(bf16).reshape(8, 128, 4, 8, 128).transpose(1, 2, 3, 0, 4)
    wur = np.ascontiguousarray(wp4[:, 0])                  # [128, 8, 8, 128]
    wvr = np.ascontiguousarray(
        wp4[:, 1].reshape(128, 2, 4, 8, 128).transpose(0, 1, 3, 2, 4)
        .reshape(128, 2, 8, 512))
    wqr = np.ascontiguousarray(wp4[:, 2])
    wkr = np.ascontiguousarray(wp4[:, 3])

    shared = dict(
        wur=wur, wvr=wvr, wqr=wqr, wkr=wkr,
        wout=W_out.astype(bf16),
        p2=p2.astype(bf16),
        bcolT=bcolT, gbT=gbT,
        vbias=np.ascontiguousarray(vbias),
        boutb=np.ascontiguousarray(boutb),
        ones128=np.ones((128, 1), np.float32),
        onesrow=np.ones((1, 128), np.float32),
    )

    exts = (EXT0, EXT1)
    in_maps = []
    for cid in range(NCORES):
        b, g = divmod(cid, 2)
        tiles = TILES0 if g == 0 else TILES1
        own = np.concatenate([np.arange(128 * t, 128 * (t + 1))
                              for t in tiles])
        xb = x[b]
        xqc = np.ascontiguousarray(xb[own])
        half = slice(OWN * g, OWN * (g + 1))
        m = dict(shared)
        m["xkvT"] = np.ascontiguousarray(xb[half].T).astype(bf16)
        m["cosk"] = np.ascontiguousarray(cosk[:, half]).astype(bf16)
        m["sink"] = np.ascontiguousarray(sink[:, half]).astype(bf16)
        m["xqT"] = np.ascontiguousarray(xqc.T).astype(bf16)
        m["xq"] = xqc
        m["cosq"] = np.ascontiguousarray(cosk[:, own]).astype(bf16)
        m["sinq"] = np.ascontiguousarray(sink[:, own]).astype(bf16)
        am = attn_mask[b]
        mk = np.zeros((2, 128, 8, 128), dtype=np.float32)
        for uu in range(2):
            for p in range(4):
                t = tiles[4 * uu + p]
                qrows = slice(128 * t, 128 * (t + 1))
                for w01 in range(2):
                    kb = exts[uu][p] - 1 - w01
                    mk[uu, :, 2 * p + w01, :] = \
                        am[qrows, 128 * kb:128 * (kb + 1)].T
        m["maskT"] = mk.astype(bf16)
        in_maps.append(m)
    return in_maps


def kernel(**inputs):
    if "nc" not in _CACHED:
        _CACHED["nc"] = _build()
    nc = _CACHED["nc"]
    in_maps = _host_prep(**inputs)
    res = run_bass_kernel_spmd(nc, in_maps, list(range(NCORES)))
    globals()["_LAST_RESULTS"] = res
    full = np.empty((B, L, D), dtype=np.float32)
    for cid in range(NCORES):
        b, g = divmod(cid, 2)
        tiles = TILES0 if g == 0 else TILES1
        o = res.results[cid]["out"]
        for i, t in enumerate(tiles):
            full[b, 128 * t:128 * (t + 1)] = o[128 * i:128 * (i + 1)]
    return full
